# revision 1
# baseline (speedup 1.0000x reference)
"""Embedding lookup (gather) on 8 Trainium2 NeuronCores.

Strategy: data-parallel. The [768, 50257] table is transposed host-side to
row-major [50257, 768] and replicated to every core's DRAM; the 8*2048 = 16384
token indices are sharded 2048 per core. Each core gathers its 2048 embedding
rows from its local table copy with indirect DMA (SWDGE) into SBUF, then
streams them out to its output shard with HWDGE stores. No collectives needed.

Per-core HBM traffic: ~6.3 MB gather read + ~6.3 MB store write -> the kernel
is DMA/HBM-roofline bound (~36 us at ~358 GB/s per-core HBM).
"""

import numpy as np

VOCAB = 50257
EMBED = 768
BATCH = 8
SEQ = 2048
N_CORES = 8
P = 128                      # SBUF partitions
TOK_PER_CORE = BATCH * SEQ // N_CORES   # 2048
GROUPS = TOK_PER_CORE // P              # 16 gather groups of 128 rows

_cached = {}
LAST_RESULTS = None  # BassKernelResults of the most recent run (for test harness)


def _build():
    """Build + compile the single-core Bass program (shared SPMD across 8 cores)."""
    import concourse.bacc as bacc
    import concourse.bass as bass
    import concourse.tile as tile
    from concourse import mybir

    nc = bacc.Bacc(
        "TRN2",
        target_bir_lowering=False,
        debug=False,
        num_devices=N_CORES,
    )

    table = nc.dram_tensor(
        "table", [VOCAB, EMBED], mybir.dt.float32, kind="ExternalInput"
    ).ap()
    idx = nc.dram_tensor(
        "idx", [P, GROUPS], mybir.dt.int32, kind="ExternalInput"
    ).ap()
    out = nc.dram_tensor(
        "out", [GROUPS, P, EMBED], mybir.dt.float32, kind="ExternalOutput"
    ).ap()

    with tile.TileContext(nc) as tc:
        with (
            tc.tile_pool(name="idxp", bufs=1) as idxp,
            tc.tile_pool(name="emb", bufs=4) as embp,
        ):
            idx_sb = idxp.tile([P, GROUPS], mybir.dt.int32)
            nc.sync.dma_start(idx_sb[:], idx[:])
            for g in range(GROUPS):
                t = embp.tile([P, EMBED], mybir.dt.float32, tag="emb")
                nc.gpsimd.indirect_dma_start(
                    out=t[:],
                    out_offset=None,
                    in_=table[:],
                    in_offset=bass.IndirectOffsetOnAxis(
                        ap=idx_sb[:, g : g + 1], axis=0
                    ),
                )
                nc.sync.dma_start(out[g], t[:])

    nc.compile()
    return nc


def kernel(x, weight):
    global LAST_RESULTS
    from concourse.bass_utils import run_bass_kernel_spmd

    if "nc" not in _cached:
        _cached["nc"] = _build()
    nc = _cached["nc"]

    # Host-side input staging: transpose table to row-major [V, D]; shard
    # tokens 2048/core, laid out [128 partitions, 16 groups] so group g of
    # core c covers tokens c*2048 + g*128 + p.
    wt = np.ascontiguousarray(np.asarray(weight, dtype=np.float32).T)
    x_flat = np.asarray(x, dtype=np.int32).reshape(N_CORES, TOK_PER_CORE)
    in_maps = []
    for c in range(N_CORES):
        idx_c = np.ascontiguousarray(x_flat[c].reshape(GROUPS, P).T)
        in_maps.append({"table": wt, "idx": idx_c})

    res = run_bass_kernel_spmd(nc, in_maps, core_ids=list(range(N_CORES)))
    LAST_RESULTS = res

    out = np.empty((N_CORES, TOK_PER_CORE, EMBED), dtype=np.float32)
    for c in range(N_CORES):
        out[c] = np.asarray(res.results[c]["out"]).reshape(TOK_PER_CORE, EMBED)
    return out.reshape(BATCH, SEQ, EMBED)


# revision 2
# speedup vs baseline: 1.2045x; 1.2045x over previous
"""Embedding lookup (gather) on 8 Trainium2 NeuronCores.

Strategy: data-parallel. The [768, 50257] table is transposed host-side to
row-major [50257, 768] and replicated to every core's DRAM; the 8*2048 = 16384
token indices are sharded 2048 per core. Each core gathers its 2048 embedding
rows from its local table copy with indirect DMA (SWDGE) into SBUF, then
streams them out to its output shard with HWDGE stores. No collectives needed.

Raw Bass (no TileContext): the Tile preamble/tail barriers cost ~16 us on a
~40 us kernel, so semaphores are managed manually. All 16 gather groups are
fully buffered in SBUF (48 KB/partition), so gathers never wait on stores.

Per-core HBM traffic: ~6.3 MB gather read + ~6.3 MB store write -> the kernel
is DMA/HBM-roofline bound.
"""

import numpy as np

VOCAB = 50257
EMBED = 768
BATCH = 8
SEQ = 2048
N_CORES = 8
P = 128                      # SBUF partitions
TOK_PER_CORE = BATCH * SEQ // N_CORES   # 2048
GROUPS = TOK_PER_CORE // P              # 16 gather groups of 128 rows

_cached = {}
LAST_RESULTS = None  # BassKernelResults of the most recent run (for test harness)


def _build():
    """Build + compile the single-core Bass program (shared SPMD across 8 cores)."""
    import concourse.bacc as bacc
    import concourse.bass as bass
    from concourse import mybir

    nc = bacc.Bacc(
        "TRN2",
        target_bir_lowering=False,
        debug=False,
        num_devices=N_CORES,
    )

    table = nc.dram_tensor(
        "table", [VOCAB, EMBED], mybir.dt.float32, kind="ExternalInput"
    ).ap()
    idx = nc.dram_tensor(
        "idx", [P, GROUPS], mybir.dt.int32, kind="ExternalInput"
    ).ap()
    out = nc.dram_tensor(
        "out", [GROUPS, P, EMBED], mybir.dt.float32, kind="ExternalOutput"
    ).ap()

    with (
        nc.sbuf_tensor("idx_sb", [P, GROUPS], mybir.dt.int32) as idx_sb,
        nc.sbuf_tensor("emb", [P, GROUPS * EMBED], mybir.dt.float32) as emb,
        nc.semaphore("gsem") as gsem,
        nc.semaphore("ssem") as ssem,
        nc.Block() as block,
    ):

        @block.gpsimd
        def _(g):
            # idx load + all gathers on the same SWDGE queue: gsem counts
            # whole-DMA completions in issue order (16 incs per DMA).
            g.dma_start(idx_sb[:], idx[:]).then_inc(gsem, 16)
            g.wait_ge(gsem, 16)
            for i in range(GROUPS):
                g.indirect_dma_start(
                    out=emb[:, i * EMBED : (i + 1) * EMBED],
                    out_offset=None,
                    in_=table[:],
                    in_offset=bass.IndirectOffsetOnAxis(
                        ap=idx_sb[:, i : i + 1], axis=0
                    ),
                ).then_inc(gsem, 16)

        @block.sync
        def _(s):
            for i in range(GROUPS):
                s.wait_ge(gsem, (i + 2) * 16)
                s.dma_start(out[i], emb[:, i * EMBED : (i + 1) * EMBED]).then_inc(
                    ssem, 16
                )
            s.wait_ge(ssem, GROUPS * 16)

    nc.compile()
    return nc


def kernel(x, weight):
    global LAST_RESULTS
    from concourse.bass_utils import run_bass_kernel_spmd

    if "nc" not in _cached:
        _cached["nc"] = _build()
    nc = _cached["nc"]

    # Host-side input staging: transpose table to row-major [V, D]; shard
    # tokens 2048/core, laid out [128 partitions, 16 groups] so group g of
    # core c covers tokens c*2048 + g*128 + p.
    wt = np.ascontiguousarray(np.asarray(weight, dtype=np.float32).T)
    x_flat = np.asarray(x, dtype=np.int32).reshape(N_CORES, TOK_PER_CORE)
    in_maps = []
    for c in range(N_CORES):
        idx_c = np.ascontiguousarray(x_flat[c].reshape(GROUPS, P).T)
        in_maps.append({"table": wt, "idx": idx_c})

    res = run_bass_kernel_spmd(nc, in_maps, core_ids=list(range(N_CORES)))
    LAST_RESULTS = res

    out = np.empty((N_CORES, TOK_PER_CORE, EMBED), dtype=np.float32)
    for c in range(N_CORES):
        out[c] = np.asarray(res.results[c]["out"]).reshape(TOK_PER_CORE, EMBED)
    return out.reshape(BATCH, SEQ, EMBED)


# revision 4
# speedup vs baseline: 1.2386x; 1.0283x over previous
"""Embedding lookup (gather) on 8 Trainium2 NeuronCores.

Strategy: data-parallel. The [768, 50257] table is transposed host-side to
row-major [50257, 768] and replicated to every core's DRAM; the 8*2048 = 16384
token indices are sharded 2048 per core. Each core gathers its 2048 embedding
rows from its local table copy with indirect DMA (SWDGE) into SBUF, then
streams them out to its output shard with HWDGE stores. No collectives needed.

Raw Bass (no TileContext, no nc.Block): all-engine barriers cost ~3-4 us each
on a ~40 us kernel, so the init barrier + const memsets are stripped from the
module and engine streams are left unsynchronized except for the DMA
semaphores that express real data dependencies:
  - SP loads the indices (HWDGE, isem), then stores each gathered group
    (ssem), alternating with ACT's HWDGE ring for the even groups.
  - Pool/GpSimd (SWDGE) waits isem, then issues the 16 indirect gathers
    back-to-back (gsem); all 16 groups are fully buffered in SBUF
    (48 KB/partition), so gathers never wait on stores.
  - Store i waits gsem >= 16*(i+1); SP's final wait on ssem covers all
    stores on both rings before the program retires.

Per-core HBM traffic: ~6.3 MB gather read + ~6.3 MB store write -> the kernel
is DMA/HBM-roofline bound.
"""

import numpy as np

VOCAB = 50257
EMBED = 768
BATCH = 8
SEQ = 2048
N_CORES = 8
P = 128                      # SBUF partitions
TOK_PER_CORE = BATCH * SEQ // N_CORES   # 2048
GROUPS = TOK_PER_CORE // P              # 16 gather groups of 128 rows

_cached = {}
LAST_RESULTS = None  # BassKernelResults of the most recent run (for test harness)


def _build():
    """Build + compile the single-core Bass program (shared SPMD across 8 cores)."""
    import concourse.bacc as bacc
    import concourse.bass as bass
    from concourse import mybir

    nc = bacc.Bacc(
        "TRN2",
        target_bir_lowering=False,
        debug=False,
        num_devices=N_CORES,
    )

    # Drop the init-time const memsets and the all-engine barrier (~3.5 us):
    # nothing in this kernel reads the const APs, and the engine streams only
    # communicate through DMA semaphores which the loader zero-initializes.
    main_blk = nc.m.functions[0].blocks[0]
    removable = [
        inst
        for inst in main_blk.instructions
        if type(inst).__name__ in ("InstMemset", "InstDrain", "InstEventSemaphore")
    ]
    for inst in removable:
        main_blk.instructions.remove(inst)

    table = nc.dram_tensor(
        "table", [VOCAB, EMBED], mybir.dt.float32, kind="ExternalInput"
    ).ap()
    idx = nc.dram_tensor(
        "idx", [P, GROUPS], mybir.dt.int32, kind="ExternalInput"
    ).ap()
    out = nc.dram_tensor(
        "out", [GROUPS, P, EMBED], mybir.dt.float32, kind="ExternalOutput"
    ).ap()

    import contextlib

    with contextlib.ExitStack() as ctx:
        idx_sb = ctx.enter_context(
            nc.sbuf_tensor("idx_sb", [P, GROUPS], mybir.dt.int32)
        )
        emb = ctx.enter_context(
            nc.sbuf_tensor("emb", [P, GROUPS * EMBED], mybir.dt.float32)
        )
        isem = ctx.enter_context(nc.semaphore("isem"))
        ssem = ctx.enter_context(nc.semaphore("ssem"))
        # One completion sem PER gather: a single SWDGE DMA's 16 increments
        # come from 16 independently-progressing SDMA engines, so cumulative
        # counts across DMAs on one sem do NOT imply per-DMA completion
        # (engine A can contribute several increments while engine B still
        # drains an earlier DMA). Same convention Tile uses (DMASWx lanes).
        gsems = [
            ctx.enter_context(nc.semaphore(f"gsem{i}")) for i in range(GROUPS)
        ]

        # SP: index load first (HWDGE - cheap descriptor gen, Q7 stays free).
        nc.sync.dma_start(idx_sb[:], idx[:]).then_inc(isem, 16)

        # Pool/SWDGE: 16 indirect gathers, fully buffered, no store waits.
        nc.gpsimd.wait_ge(isem, 16)
        for i in range(GROUPS):
            nc.gpsimd.indirect_dma_start(
                out=emb[:, i * EMBED : (i + 1) * EMBED],
                out_offset=None,
                in_=table[:],
                in_offset=bass.IndirectOffsetOnAxis(ap=idx_sb[:, i : i + 1], axis=0),
            ).then_inc(gsems[i], 16)

        # Stores: alternate the two HWDGE rings (SP=qSPDynamicHW,
        # ACT=qActDynamicHW) so more store packets are in flight per SDMA
        # engine while gather packets round-robin on the SWDGE ring.
        for i in range(GROUPS):
            eng = nc.sync if i % 2 == 0 else nc.scalar
            eng.wait_ge(gsems[i], 16)
            eng.dma_start(out[i], emb[:, i * EMBED : (i + 1) * EMBED]).then_inc(
                ssem, 16
            )

        # All stores landed (sem increments fire after last-byte receipt).
        # A cumulative wait is sound here: GROUPS*16 is the maximum total.
        nc.sync.wait_ge(ssem, GROUPS * 16)

    nc.compile()
    return nc


def kernel(x, weight):
    global LAST_RESULTS
    from concourse.bass_utils import run_bass_kernel_spmd

    if "nc" not in _cached:
        _cached["nc"] = _build()
    nc = _cached["nc"]

    # Host-side input staging: transpose table to row-major [V, D]; shard
    # tokens 2048/core, laid out [128 partitions, 16 groups] so group g of
    # core c covers tokens c*2048 + g*128 + p.
    wt = np.ascontiguousarray(np.asarray(weight, dtype=np.float32).T)
    x_flat = np.asarray(x, dtype=np.int32).reshape(N_CORES, TOK_PER_CORE)
    in_maps = []
    for c in range(N_CORES):
        idx_c = np.ascontiguousarray(x_flat[c].reshape(GROUPS, P).T)
        in_maps.append({"table": wt, "idx": idx_c})

    res = run_bass_kernel_spmd(nc, in_maps, core_ids=list(range(N_CORES)))
    LAST_RESULTS = res

    out = np.empty((N_CORES, TOK_PER_CORE, EMBED), dtype=np.float32)
    for c in range(N_CORES):
        out[c] = np.asarray(res.results[c]["out"]).reshape(TOK_PER_CORE, EMBED)
    return out.reshape(BATCH, SEQ, EMBED)
